# revision 4
# baseline (speedup 1.0000x reference)
"""Trainium2 Bass kernel for the HNN leapfrog dynamical-inference layer.

Reference: 3 leapfrog steps over phase space zp=[q,p], H(zp)=sum(MLP(zp)),
MLP = tanh(zp@W1+b1) -> tanh(@W2+b2) -> @W3+b3; 3 gradient evals per step.

Algebraic restructuring (validated ~5e-8 rel err vs reference in fp32):
  - p0 = 0 and q,p only enter through a = zp@W1, so track the 256-dim
    pre-activation state T = q@W1q + p@W1p instead of q,p.
  - kick:  p -= c*gq  =>  T += u1s @ (W1q^T W1p)  (Mqp precomputed)
  - drift: q += dt*gp =>  T += u1s @ (W1p^T W1q)  (Mpq precomputed)
    u1s = scale*(1-h1^2)*((1-h2^2)*w3 @ W2^T), scale folded in.
  - q_final = z + (sum over drift evals of u1s) @ W1p^T; last kick dead.
  - (1-h2^2)*w3 @ W2^T = C + h2^2 @ W2wneg, C = W2@w3,
    W2wneg[j,i] = -w3[j]*W2[i,j]. The integration scale is folded into
    per-eval-kind copies of W2wneg and C on the host; C enters the PSUM
    accumulation as a rank-1 matmul (C x ones) so the whole u2/v stage
    costs zero elementwise ops.
  - u1s = (h1^2 - 1) * v computed as one scalar_tensor_tensor reading the
    backward matmul PSUM directly.
All matmuls bf16 (full PE rate); T and the final q = z + s@W1pT add stay
fp32. The gradient path enters the output scaled by ~0.006, so bf16 there
costs ~2.5e-5 end-to-end relative error.

Layout: activations transposed (features on partitions, batch on free
axis); weights host-pretransposed so no on-device transposes exist.
Elementwise ops run on [128, 1024] tiles (2 halves per core) to amortize
the per-op overhead (~352 cycles on ACT); matmuls slice 512-wide chunks.
Sharding: pure data parallel, batch 16384 -> 8 cores x 2048 rows.
"""

import numpy as np
import ml_dtypes

import concourse.mybir as mybir
import concourse.tile as tile
from concourse import bacc
from concourse.bass_utils import run_bass_kernel_spmd

AF = mybir.ActivationFunctionType
ALU = mybir.AluOpType
FP32 = mybir.dt.float32
BF16 = mybir.dt.bfloat16
BF = ml_dtypes.bfloat16

N_CORES = 8
B, DIM, HID = 16384, 512, 256
DT = 0.1
BL = B // N_CORES            # batch rows per core (2048)
NH = 2                       # elementwise halves per core
EW = BL // NH                # elementwise tile width (1024)
NN = EW // 512               # matmul n-slices per half (2)
KD = DIM // 128              # k-tiles over q-features (4)
KH = HID // 128              # k-tiles over hidden (2)
MQ = DIM // 128              # m-tiles over final q-features (4)

EVALS = ["k", "d", "k", "k", "d", "k", "k", "d"]


def build_nc():
    nc = bacc.Bacc("TRN2", target_bir_lowering=False, debug=False)

    zT_d = nc.dram_tensor("zT", [DIM, BL], FP32, kind="ExternalInput")
    zTb_d = nc.dram_tensor("zTb", [DIM, BL], BF16, kind="ExternalInput")
    w1q_d = nc.dram_tensor("w1q", [128, KD, HID], BF16, kind="ExternalInput")
    w2_d = nc.dram_tensor("w2", [128, KH, HID], BF16, kind="ExternalInput")
    wk_d = nc.dram_tensor("wk", [128, KH, HID], BF16, kind="ExternalInput")
    wd_d = nc.dram_tensor("wd", [128, KH, HID], BF16, kind="ExternalInput")
    mqp_d = nc.dram_tensor("mqp", [128, KH, HID], BF16, kind="ExternalInput")
    mpq_d = nc.dram_tensor("mpq", [128, KH, HID], BF16, kind="ExternalInput")
    w1pt_d = nc.dram_tensor("w1pt", [128, KH, DIM], BF16, kind="ExternalInput")
    b1_d = nc.dram_tensor("b1", [128, KH], FP32, kind="ExternalInput")
    b2_d = nc.dram_tensor("b2", [128, KH], FP32, kind="ExternalInput")
    ckr_d = nc.dram_tensor("ckr", [1, HID], BF16, kind="ExternalInput")
    cdr_d = nc.dram_tensor("cdr", [1, HID], BF16, kind="ExternalInput")
    qT_d = nc.dram_tensor("qT", [DIM, BL], FP32, kind="ExternalOutput")

    with tile.TileContext(nc) as tc:
        with (
            tc.tile_pool(name="const", bufs=1) as cp,
            tc.tile_pool(name="state", bufs=1) as sp,
            tc.tile_pool(name="work", bufs=1) as wp,
            tc.tile_pool(name="qo", bufs=4) as qp,
            tc.tile_pool(name="ps", bufs=4, space="PSUM") as pp,
        ):
            # ---- weights / biases
            w1q = cp.tile([128, KD, HID], BF16, tag="w1q", name="w1q")
            nc.sync.dma_start(w1q[:], w1q_d.ap()[:])
            w2 = cp.tile([128, KH, HID], BF16, tag="w2", name="w2")
            nc.sync.dma_start(w2[:], w2_d.ap()[:])
            wk = cp.tile([128, KH, HID], BF16, tag="wk", name="wk")
            nc.sync.dma_start(wk[:], wk_d.ap()[:])
            wd = cp.tile([128, KH, HID], BF16, tag="wd", name="wd")
            nc.sync.dma_start(wd[:], wd_d.ap()[:])
            mqp = cp.tile([128, KH, HID], BF16, tag="mqp", name="mqp")
            nc.sync.dma_start(mqp[:], mqp_d.ap()[:])
            mpq = cp.tile([128, KH, HID], BF16, tag="mpq", name="mpq")
            nc.sync.dma_start(mpq[:], mpq_d.ap()[:])
            w1pt = cp.tile([128, KH, DIM], BF16, tag="w1pt", name="w1pt")
            nc.sync.dma_start(w1pt[:], w1pt_d.ap()[:])
            b1 = cp.tile([128, KH], FP32, tag="b1", name="b1")
            nc.sync.dma_start(b1[:], b1_d.ap()[:])
            b2 = cp.tile([128, KH], FP32, tag="b2", name="b2")
            nc.sync.dma_start(b2[:], b2_d.ap()[:])
            ckr = cp.tile([1, HID], BF16, tag="ckr", name="ckr")
            nc.sync.dma_start(ckr[:], ckr_d.ap()[:])
            cdr = cp.tile([1, HID], BF16, tag="cdr", name="cdr")
            nc.sync.dma_start(cdr[:], cdr_d.ap()[:])
            ones = cp.tile([1, 512], BF16, tag="ones", name="ones")
            nc.vector.memset(ones[:], 1.0)

            # ---- batch-resident inputs
            zTb = [
                sp.tile([128, BL], BF16, tag=f"zTb{k}", name=f"zTb{k}")
                for k in range(KD)
            ]
            for k in range(KD):
                nc.sync.dma_start(zTb[k][:], zTb_d.ap()[k * 128 : (k + 1) * 128, :])

            # ---- persistent per-half state
            T = [
                [
                    sp.tile([128, EW], FP32, tag=f"T{h}_{m}", name=f"T{h}_{m}")
                    for m in range(KH)
                ]
                for h in range(NH)
            ]
            s = [
                [
                    sp.tile([128, EW], BF16, tag=f"s{h}_{m}", name=f"s{h}_{m}")
                    for m in range(KH)
                ]
                for h in range(NH)
            ]

            def hsl(h, n):  # batch cols of n-slice inside half h
                return slice(h * EW + n * 512, h * EW + (n + 1) * 512)

            # ---- init: T = z @ W1q (p0 = 0)
            for h in range(NH):
                for m in range(KH):
                    ps = pp.tile([128, EW], FP32, tag="mm", name="mm")
                    for n in range(NN):
                        for k in range(KD):
                            nc.tensor.matmul(
                                ps[:, n * 512 : (n + 1) * 512],
                                w1q[:, k, m * 128 : (m + 1) * 128],
                                zTb[k][:, hsl(h, n)],
                                start=(k == 0),
                                stop=(k == KD - 1),
                            )
                    nc.scalar.activation(T[h][m][:], ps[:], AF.Copy)

            # fp32 z arrives during the eval chain; only needed at the end
            zT = [
                sp.tile([128, BL], FP32, tag=f"zT{k}", name=f"zT{k}")
                for k in range(KD)
            ]
            for k in range(KD):
                nc.sync.dma_start(zT[k][:], zT_d.ap()[k * 128 : (k + 1) * 128, :])

            # ---- 8 gradient evals
            for ei, kind in enumerate(EVALS):
                wv = wk if kind == "k" else wd
                cr = ckr if kind == "k" else cdr
                updw = mqp if kind == "k" else mpq
                first_drift = kind == "d" and ei == 1
                is_last = ei == len(EVALS) - 1

                h1 = [
                    [
                        wp.tile([128, EW], BF16, tag=f"h1_{h}_{m}", name=f"h1_{h}_{m}")
                        for m in range(KH)
                    ]
                    for h in range(NH)
                ]
                sq1 = [
                    [
                        wp.tile([128, EW], BF16, tag=f"sq1_{h}_{m}", name=f"sq1_{h}_{m}")
                        for m in range(KH)
                    ]
                    for h in range(NH)
                ]
                h2 = [
                    [
                        wp.tile([128, EW], BF16, tag=f"h2_{h}_{m}", name=f"h2_{h}_{m}")
                        for m in range(KH)
                    ]
                    for h in range(NH)
                ]
                sq2 = [
                    [
                        wp.tile([128, EW], BF16, tag=f"sq2_{h}_{m}", name=f"sq2_{h}_{m}")
                        for m in range(KH)
                    ]
                    for h in range(NH)
                ]
                u1 = [
                    [
                        wp.tile([128, EW], BF16, tag=f"u1_{h}_{m}", name=f"u1_{h}_{m}")
                        for m in range(KH)
                    ]
                    for h in range(NH)
                ]

                for h in range(NH):
                    # h1 = tanh(T + b1); derivative needs h1^2 (split ACT/DVE)
                    for m in range(KH):
                        nc.scalar.activation(
                            h1[h][m][:], T[h][m][:], AF.Tanh, bias=b1[:, m : m + 1]
                        )
                    nc.scalar.activation(sq1[h][0][:], h1[h][0][:], AF.Square)
                    nc.vector.tensor_mul(sq1[h][1][:], h1[h][1][:], h1[h][1][:])

                    # h2 = tanh(h1 @ W2 + b2), sq2 = h2^2
                    for m in range(KH):
                        ps = pp.tile([128, EW], FP32, tag="mm", name="mm")
                        for n in range(NN):
                            for k in range(KH):
                                nc.tensor.matmul(
                                    ps[:, n * 512 : (n + 1) * 512],
                                    w2[:, k, m * 128 : (m + 1) * 128],
                                    h1[h][k][:, n * 512 : (n + 1) * 512],
                                    start=(k == 0),
                                    stop=(k == KH - 1),
                                )
                        nc.scalar.activation(
                            h2[h][m][:], ps[:], AF.Tanh, bias=b2[:, m : m + 1]
                        )
                    for m in range(KH):
                        nc.vector.tensor_mul(sq2[h][m][:], h2[h][m][:], h2[h][m][:])

                    # v = scale*(C + sq2 @ W2wneg) via pre-scaled wv + rank-1 C
                    # u1 = (sq1 - 1) * v, straight off PSUM
                    for m in range(KH):
                        ps = pp.tile([128, EW], FP32, tag="mm", name="mm")
                        for n in range(NN):
                            for k in range(KH):
                                nc.tensor.matmul(
                                    ps[:, n * 512 : (n + 1) * 512],
                                    wv[:, k, m * 128 : (m + 1) * 128],
                                    sq2[h][k][:, n * 512 : (n + 1) * 512],
                                    start=(k == 0),
                                    stop=False,
                                )
                            nc.tensor.matmul(
                                ps[:, n * 512 : (n + 1) * 512],
                                cr[:, m * 128 : (m + 1) * 128],
                                ones[:],
                                start=False,
                                stop=True,
                            )
                        nc.vector.scalar_tensor_tensor(
                            u1[h][m][:],
                            sq1[h][m][:],
                            1.0,
                            ps[:],
                            ALU.subtract,
                            ALU.mult,
                        )

                    # s accumulation on drift evals (bf16)
                    if kind == "d":
                        for m in range(KH):
                            if first_drift:
                                nc.vector.tensor_copy(s[h][m][:], u1[h][m][:])
                            else:
                                nc.vector.tensor_add(
                                    s[h][m][:], s[h][m][:], u1[h][m][:]
                                )

                    # T += u1 @ updw (dead after the last drift)
                    if not is_last:
                        for m in range(KH):
                            ps = pp.tile([128, EW], FP32, tag="mm", name="mm")
                            for n in range(NN):
                                for k in range(KH):
                                    nc.tensor.matmul(
                                        ps[:, n * 512 : (n + 1) * 512],
                                        updw[:, k, m * 128 : (m + 1) * 128],
                                        u1[h][k][:, n * 512 : (n + 1) * 512],
                                        start=(k == 0),
                                        stop=(k == KH - 1),
                                    )
                            nc.vector.tensor_add(T[h][m][:], T[h][m][:], ps[:])

            # ---- final: q = z + s @ W1p^T
            for h in range(NH):
                for mq in range(MQ):
                    ps = pp.tile([128, EW], FP32, tag="mm", name="mm")
                    for n in range(NN):
                        for k in range(KH):
                            nc.tensor.matmul(
                                ps[:, n * 512 : (n + 1) * 512],
                                w1pt[:, k, mq * 128 : (mq + 1) * 128],
                                s[h][k][:, n * 512 : (n + 1) * 512],
                                start=(k == 0),
                                stop=(k == KH - 1),
                            )
                    qo = qp.tile([128, EW], FP32, tag="qo", name="qo")
                    nc.vector.tensor_add(
                        qo[:], zT[mq][:, h * EW : (h + 1) * EW], ps[:]
                    )
                    nc.sync.dma_start(
                        qT_d.ap()[mq * 128 : (mq + 1) * 128, h * EW : (h + 1) * EW],
                        qo[:],
                    )

    nc.compile()
    return nc


_CACHE = {}


def _get_nc():
    if "nc" not in _CACHE:
        _CACHE["nc"] = build_nc()
    return _CACHE["nc"]


def _tile_k(a, ktiles):
    """[K, M] -> [128, ktiles, M] with K = ktiles*128 on partitions."""
    k, m = a.shape
    assert k == ktiles * 128
    return np.ascontiguousarray(a.reshape(ktiles, 128, m).transpose(1, 0, 2))


def _bias_tiles(v):
    """[256] -> [128, 2]: column m holds features m*128..(m+1)*128."""
    return np.ascontiguousarray(v.reshape(KH, 128).T)


def _prep_shared(W1, b1, W2, b2, W3, b3):
    W1 = np.asarray(W1, dtype=np.float32)
    W2 = np.asarray(W2, dtype=np.float32)
    w3 = np.asarray(W3, dtype=np.float32)[:, 0]
    b1 = np.asarray(b1, dtype=np.float32)
    b2 = np.asarray(b2, dtype=np.float32)
    W1q, W1p = W1[:DIM], W1[DIM:]
    W2wneg = -(w3[:, None] * W2.T)
    C = W2 @ w3
    Mqp = W1q.T @ W1p
    Mpq = W1p.T @ W1q
    # kick: u1 = (h1^2-1) * (dt/2)*(C + sq2@W2wneg)  (scale -dt/2 folded)
    # drift: u1 = (h1^2-1) * (-dt)*(C + sq2@W2wneg)  (scale +dt folded)
    return {
        "w1q": _tile_k(W1q, KD).astype(BF),
        "w2": _tile_k(W2, KH).astype(BF),
        "wk": _tile_k((DT / 2) * W2wneg, KH).astype(BF),
        "wd": _tile_k((-DT) * W2wneg, KH).astype(BF),
        "mqp": _tile_k(Mqp, KH).astype(BF),
        "mpq": _tile_k(Mpq, KH).astype(BF),
        "w1pt": _tile_k(np.ascontiguousarray(W1p.T), KH).astype(BF),
        "b1": _bias_tiles(b1),
        "b2": _bias_tiles(b2),
        "ckr": ((DT / 2) * C).reshape(1, HID).astype(BF),
        "cdr": ((-DT) * C).reshape(1, HID).astype(BF),
    }


def run_kernel(z, W1, b1, W2, b2, W3, b3, trace=False, trace_cores=None):
    nc = _get_nc()
    shared = _prep_shared(W1, b1, W2, b2, W3, b3)
    z = np.asarray(z, dtype=np.float32)
    in_maps = []
    for i in range(N_CORES):
        zt = np.ascontiguousarray(z[i * BL : (i + 1) * BL].T)
        in_maps.append({**shared, "zT": zt, "zTb": zt.astype(BF)})
    res = run_bass_kernel_spmd(
        nc,
        in_maps,
        core_ids=list(range(N_CORES)),
        trace=trace,
        trace_cores=trace_cores,
    )
    out = np.concatenate([res.results[i]["qT"].T for i in range(N_CORES)], axis=0)
    return np.ascontiguousarray(out), res


def kernel(z, W1, b1, W2, b2, W3, b3):
    out, _ = run_kernel(z, W1, b1, W2, b2, W3, b3)
    return out
